# revision 2
# baseline (speedup 1.0000x reference)
"""Bass/Trainium2 kernel for nn_DenseMap (bilinear grid-sample embedding lookup).

Strategy: shard the 128 maps across 8 NeuronCores (16 maps each, in 2 phases
of 8). Table relayout (host): per map, 8 feature columns x 2 x-shifts, each
stored as even-start and odd-start y-pair streams so ONE ap_gather index with
d=2 fetches all 4 bilinear neighbors x 8 features across the 16 SBUF
partitions owned by one GPSIMD core. Device: ap_gather + DVE lerps +
stream_shuffle + PE select-transpose merge, DMA out sample-major.
"""
import sys, os
sys.path.insert(0, "/opt/trn_rl_repo")
import numpy as np

FEAT = 8
RES = 128
OFF = RES * RES          # 16384 grid pts / map
MAPS = 128
B = 32768
NCORES = 8
MP_NC = 16               # maps per NeuronCore
PH_M = 8                 # maps per phase
S = 1024                 # samples per chunk
NCH = B // S             # 32 chunks per phase
NPAIR = OFF              # num_elems for ap_gather (8192 E-pairs + 8192 O-pairs)

_cache = {}


def _build_program():
    import concourse.bass as bass
    import concourse.tile as tile
    from concourse import bacc, mybir

    nc = bacc.Bacc("TRN2", target_bir_lowering=False, debug=False,
                   num_devices=NCORES)
    dt = mybir.dt
    emb_d = [nc.dram_tensor(f"emb{p}", [128, 2 * NPAIR], dt.float32,
                            kind="ExternalInput").ap() for p in range(2)]
    idx_d = [nc.dram_tensor(f"idx{p}", [128, (S // 16) * NCH], dt.int16,
                            kind="ExternalInput").ap() for p in range(2)]
    wxa_d = [nc.dram_tensor(f"wxa{p}", [128, B], dt.float32,
                            kind="ExternalInput").ap() for p in range(2)]
    wya_d = [nc.dram_tensor(f"wya{p}", [128, B], dt.float32,
                            kind="ExternalInput").ap() for p in range(2)]
    p1_d = nc.dram_tensor("p1", [128, 80], dt.float32, kind="ExternalInput").ap()
    p2_d = nc.dram_tensor("p2", [128, 80], dt.float32, kind="ExternalInput").ap()
    p3_d = nc.dram_tensor("p3", [128, 80], dt.float32, kind="ExternalInput").ap()
    out_d = nc.dram_tensor("out", [2, NCH, 128, (S // 128) * 80], dt.float32,
                           kind="ExternalOutput").ap()

    # stream_shuffle mask: within each 32-partition quadrant, rows 0..7 <- 8..15,
    # rows 16..23 <- 24..31 (pull the x+1-shift partitions down beside shift-0).
    mask = [8, 9, 10, 11, 12, 13, 14, 15, 8, 9, 10, 11, 12, 13, 14, 15,
            24, 25, 26, 27, 28, 29, 30, 31, 24, 25, 26, 27, 28, 29, 30, 31]

    with tile.TileContext(nc) as tc:
        from contextlib import ExitStack
        with ExitStack() as ctx:
            cpool = ctx.enter_context(tc.tile_pool(name="consts", bufs=1))
            tpool = ctx.enter_context(tc.tile_pool(name="tbl", bufs=1))
            ipool = ctx.enter_context(tc.tile_pool(name="idx", bufs=1))
            wpool = ctx.enter_context(tc.tile_pool(name="w", bufs=2))
            gpool = ctx.enter_context(tc.tile_pool(name="g", bufs=2))
            rpool = ctx.enter_context(tc.tile_pool(name="r", bufs=1))
            opool = ctx.enter_context(tc.tile_pool(name="o", bufs=2))
            fpool = ctx.enter_context(tc.tile_pool(name="ft", bufs=2, space="PSUM"))

            p1_t = cpool.tile([128, 80], dt.float32, tag="p1")
            p2_t = cpool.tile([128, 80], dt.float32, tag="p2")
            p3_t = cpool.tile([128, 80], dt.float32, tag="p3")
            nc.sync.dma_start(p1_t[:], p1_d[:])
            nc.sync.dma_start(p2_t[:], p2_d[:])
            nc.sync.dma_start(p3_t[:], p3_d[:])

            for ph in range(2):
                tbl = tpool.tile([128, 2 * NPAIR], dt.float32, tag="tbl")
                nc.sync.dma_start(tbl[:], emb_d[ph][:])
                idxt = ipool.tile([128, (S // 16) * NCH], dt.int16, tag="idx")
                nc.sync.dma_start(idxt[:], idx_d[ph][:])

                for ch in range(NCH):
                    s0 = ch * S
                    wxa = wpool.tile([128, S], dt.float32, tag="wxa")
                    wya = wpool.tile([128, S], dt.float32, tag="wya")
                    nc.sync.dma_start(wxa[:], wxa_d[ph][:, s0:s0 + S])
                    nc.sync.dma_start(wya[:], wya_d[ph][:, s0:s0 + S])

                    g = gpool.tile([128, S, 2], dt.float32, tag="g")
                    nc.gpsimd.ap_gather(
                        g[:], tbl[:].rearrange("p (n d) -> p n d", d=2),
                        idxt[:, ch * (S // 16):(ch + 1) * (S // 16)],
                        channels=128, num_elems=NPAIR, d=2, num_idxs=S)

                    g0 = g[:, :, 0]
                    g1 = g[:, :, 1]
                    dd = rpool.tile([128, S], dt.float32, tag="tmp")
                    r = rpool.tile([128, S], dt.float32, tag="r")
                    nc.vector.tensor_sub(dd[:], g1, g0)
                    nc.vector.tensor_mul(dd[:], dd[:], wya[:])
                    nc.vector.tensor_add(r[:], dd[:], g0)

                    r1 = rpool.tile([128, S], dt.float32, tag="r1")
                    nc.vector.stream_shuffle(r1[:], r[:], mask)
                    d2 = rpool.tile([128, S], dt.float32, tag="tmp2")
                    nc.vector.tensor_sub(d2[:], r1[:], r[:])
                    nc.vector.tensor_mul(d2[:], d2[:], wxa[:])
                    out8 = rpool.tile([128, S], dt.float32, tag="out8")
                    nc.vector.tensor_add(out8[:], d2[:], r[:])

                    outT = opool.tile([128, (S // 128) * 80], dt.float32, tag="outT")
                    for b in range(S // 128):
                        ft = fpool.tile([128, 80], dt.float32, tag="ft")
                        sl = slice(b * 128, (b + 1) * 128)
                        nc.tensor.matmul(ft[:], out8[:, sl], p1_t[:],
                                         start=True, stop=False)
                        nc.tensor.matmul(ft[:], wxa[:, sl], p2_t[:],
                                         start=False, stop=False)
                        nc.tensor.matmul(ft[:], wya[:, sl], p3_t[:],
                                         start=False, stop=True)
                        nc.scalar.copy(outT[:, b * 80:(b + 1) * 80], ft[:])

                    nc.sync.dma_start(out_d[ph, ch], outT[:])
    nc.compile()
    return nc


def _prep_tables(embeddings):
    # [128 maps, 16516 padded grid pts, 8 feats]
    T = np.pad(embeddings.reshape(MAPS, OFF, FEAT).astype(np.float32),
               ((0, 0), (0, 132), (0, 0)))
    v = np.arange(2 * NPAIR)
    j = v // 2
    h = v % 2
    sec = j // (NPAIR // 2)
    jj = j % (NPAIR // 2)
    colg = 2 * jj + h + sec                       # [32768]
    pp = np.arange(128)
    cm = pp // 16
    sh = (pp % 16) // 8
    ff = pp % 8
    gidx = colg[None, :] + 128 * sh[:, None]      # [128, 32768]
    tabs = []
    for k in range(NCORES):
        per_ph = []
        for ph in range(2):
            maps = np.arange(16 * k + 8 * ph, 16 * k + 8 * ph + 8)
            tab = T[maps[cm][:, None], gidx, ff[:, None]]
            per_ph.append(np.ascontiguousarray(tab, dtype=np.float32))
        tabs.append(per_ph)
    return tabs


def _prep_idx_weights(inputs):
    # inputs [B, 128, 2] -> per NC, per phase: idx [128, 64*NCH] i16,
    # wxa/wya [128, B] f32 (replicated across each 16-partition group)
    x = (inputs[..., 0].astype(np.float32) * np.float32(RES - 1)).astype(np.float32)
    y = (inputs[..., 1].astype(np.float32) * np.float32(RES - 1)).astype(np.float32)
    xi = x.astype(np.int32).astype(np.int64)          # trunc, matches reference
    yi = y.astype(np.int32).astype(np.int64)
    xf = x - xi.astype(np.float32)
    yf = y - yi.astype(np.float32)
    gg = xi * RES + yi
    par = yi & 1
    idx = ((gg - par) >> 1) + par * (NPAIR // 2)  # [B, 128] int
    out = []
    for k in range(NCORES):
        per_ph = []
        for ph in range(2):
            m0 = 16 * k + 8 * ph
            idx_m = idx[:, m0:m0 + 8]             # [B, 8]
            # wrapped: partition 16m+jj slot (ch*64 + t) <- sample 16*t+jj of chunk ch
            iw = idx_m.reshape(NCH, 64, 16, 8)    # [ch, t, jj, m]
            iw = iw.transpose(3, 2, 0, 1).reshape(8, 16, NCH * 64)
            iw = iw.reshape(128, NCH * 64, order="C")  # p = m*16+jj
            wx = np.repeat(xf[:, m0:m0 + 8].T, 16, axis=0)  # [128, B]
            wy = np.repeat(yf[:, m0:m0 + 8].T, 16, axis=0)
            per_ph.append((np.ascontiguousarray(iw.astype(np.int16)),
                           np.ascontiguousarray(wx),
                           np.ascontiguousarray(wy)))
        out.append(per_ph)
    return out


def _selectors():
    p1 = np.zeros((128, 80), np.float32)
    p2 = np.zeros((128, 80), np.float32)
    p3 = np.zeros((128, 80), np.float32)
    for p in range(128):
        m, q = p // 16, p % 16
        if q < 8:
            p1[p, m * 10 + q] = 1.0
        if q == 0:
            p2[p, m * 10 + 8] = 1.0
            p3[p, m * 10 + 9] = 1.0
    return p1, p2, p3


def build_in_maps(inputs: np.ndarray, embeddings: np.ndarray):
    if "nc" not in _cache:
        _cache["nc"] = _build_program()
    nc = _cache["nc"]

    inputs = np.asarray(inputs, dtype=np.float32)
    embeddings = np.asarray(embeddings, dtype=np.float32)
    tabs = _prep_tables(embeddings)
    iw = _prep_idx_weights(inputs)
    p1, p2, p3 = _selectors()

    in_maps = []
    for k in range(NCORES):
        m = {"p1": p1, "p2": p2, "p3": p3}
        for ph in range(2):
            ix, wx, wy = iw[k][ph]
            m[f"emb{ph}"] = tabs[k][ph]
            m[f"idx{ph}"] = ix
            m[f"wxa{ph}"] = wx
            m[f"wya{ph}"] = wy
        in_maps.append(m)
    return nc, in_maps


def kernel(inputs: np.ndarray, embeddings: np.ndarray) -> np.ndarray:
    from concourse.bass_utils import run_bass_kernel_spmd

    nc, in_maps = build_in_maps(inputs, embeddings)
    res = run_bass_kernel_spmd(nc, in_maps, core_ids=list(range(NCORES)))
    out = np.empty((B, MAPS, FEAT + 2), np.float32)
    for k in range(NCORES):
        st = res.results[k]["out"].reshape(2, NCH, 128, S // 128, PH_M, FEAT + 2)
        # out[ch*S + b*128 + p, 16k + ph*8 + m, f] = st[ph, ch, p, b, m, f]
        o = st.transpose(1, 3, 2, 0, 4, 5).reshape(B, MP_NC, FEAT + 2)
        out[:, 16 * k:16 * k + 16, :] = o
    return out



# revision 10
# speedup vs baseline: 1.1081x; 1.1081x over previous
"""Bass/Trainium2 kernel for nn_DenseMap (bilinear grid-sample embedding lookup).

Strategy: shard the 128 maps across 8 NeuronCores (16 maps each, in 2 phases
of 8). Table relayout (host): per map, 8 feature columns x 2 x-shifts, each
stored as even-start and odd-start y-pair streams so ONE ap_gather index with
d=2 fetches all 4 bilinear neighbors x 8 features across the 16 SBUF
partitions owned by one GPSIMD core. Device: ap_gather + DVE lerps +
stream_shuffle + PE select-transpose merge, DMA out sample-major.
"""
import sys, os
sys.path.insert(0, "/opt/trn_rl_repo")
import numpy as np

FEAT = 8
RES = 128
OFF = RES * RES          # 16384 grid pts / map
MAPS = 128
B = 32768
NCORES = 8
MP_NC = 16               # maps per NeuronCore
PH_M = 8                 # maps per phase
S = 1024                 # samples per chunk
NCH = B // S             # 32 chunks per phase
NPAIR = OFF              # num_elems for ap_gather (8192 E-pairs + 8192 O-pairs)

_cache = {}


def _build_program():
    import concourse.bass as bass
    import concourse.tile as tile
    from concourse import bacc, mybir

    nc = bacc.Bacc("TRN2", target_bir_lowering=False, debug=False,
                   num_devices=NCORES)
    dt = mybir.dt
    emb_d = [nc.dram_tensor(f"emb{p}", [128, 2 * NPAIR], dt.float32,
                            kind="ExternalInput").ap() for p in range(2)]
    idx_d = [nc.dram_tensor(f"idx{p}", [128, (S // 16) * NCH], dt.int16,
                            kind="ExternalInput").ap() for p in range(2)]
    wxa_d = [nc.dram_tensor(f"wxa{p}", [128, B], dt.float32,
                            kind="ExternalInput").ap() for p in range(2)]
    wya_d = [nc.dram_tensor(f"wya{p}", [128, B], dt.float32,
                            kind="ExternalInput").ap() for p in range(2)]
    p1_d = nc.dram_tensor("p1", [128, 80], dt.float32, kind="ExternalInput").ap()
    p2_d = nc.dram_tensor("p2", [128, 80], dt.float32, kind="ExternalInput").ap()
    p3_d = nc.dram_tensor("p3", [128, 80], dt.float32, kind="ExternalInput").ap()
    out_d = nc.dram_tensor("out", [2, NCH, 128, (S // 128) * 80], dt.float32,
                           kind="ExternalOutput").ap()

    # stream_shuffle mask: within each 32-partition quadrant, rows 0..7 <- 8..15,
    # rows 16..23 <- 24..31 (pull the x+1-shift partitions down beside shift-0).
    mask = [8, 9, 10, 11, 12, 13, 14, 15, 8, 9, 10, 11, 12, 13, 14, 15,
            24, 25, 26, 27, 28, 29, 30, 31, 24, 25, 26, 27, 28, 29, 30, 31]

    with tile.TileContext(nc) as tc:
        from contextlib import ExitStack
        with ExitStack() as ctx:
            cpool = ctx.enter_context(tc.tile_pool(name="consts", bufs=1))
            tpool = ctx.enter_context(tc.tile_pool(name="tbl", bufs=1))
            ipool = ctx.enter_context(tc.tile_pool(name="idx", bufs=1))
            wpool = ctx.enter_context(tc.tile_pool(name="w", bufs=2))
            gpool = ctx.enter_context(tc.tile_pool(name="g", bufs=2))
            rpool = ctx.enter_context(tc.tile_pool(name="r", bufs=1))
            opool = ctx.enter_context(tc.tile_pool(name="o", bufs=2))
            fpool = ctx.enter_context(tc.tile_pool(name="ft", bufs=2, space="PSUM"))

            p1_t = cpool.tile([128, 80], dt.float32, tag="p1")
            p2_t = cpool.tile([128, 80], dt.float32, tag="p2")
            p3_t = cpool.tile([128, 80], dt.float32, tag="p3")
            nc.sync.dma_start(p1_t[:], p1_d[:])
            nc.sync.dma_start(p2_t[:], p2_d[:])
            nc.sync.dma_start(p3_t[:], p3_d[:])

            for ph in range(2):
                tbl = tpool.tile([128, 2 * NPAIR], dt.float32, tag="tbl")
                nc.sync.dma_start(tbl[:], emb_d[ph][:])
                idxt = ipool.tile([128, (S // 16) * NCH], dt.int16, tag="idx")
                nc.sync.dma_start(idxt[:], idx_d[ph][:])

                for ch in range(NCH):
                    s0 = ch * S
                    wxa = wpool.tile([128, S], dt.float32, tag="wxa")
                    wya = wpool.tile([128, S], dt.float32, tag="wya")
                    nc.sync.dma_start(wxa[:], wxa_d[ph][:, s0:s0 + S])
                    nc.sync.dma_start(wya[:], wya_d[ph][:, s0:s0 + S])

                    g = gpool.tile([128, S, 2], dt.float32, tag="g")
                    nc.gpsimd.ap_gather(
                        g[:], tbl[:].rearrange("p (n d) -> p n d", d=2),
                        idxt[:, ch * (S // 16):(ch + 1) * (S // 16)],
                        channels=128, num_elems=NPAIR, d=2, num_idxs=S)

                    g0 = g[:, :, 0]
                    g1 = g[:, :, 1]
                    dd = rpool.tile([128, S], dt.float32, tag="tmp")
                    r = rpool.tile([128, S], dt.float32, tag="r")
                    nc.vector.tensor_sub(dd[:], g1, g0)
                    nc.vector.tensor_mul(dd[:], dd[:], wya[:])
                    nc.vector.tensor_add(r[:], dd[:], g0)

                    r1 = rpool.tile([128, S], dt.float32, tag="r1")
                    nc.vector.stream_shuffle(r1[:], r[:], mask)
                    d2 = rpool.tile([128, S], dt.float32, tag="tmp2")
                    nc.vector.tensor_sub(d2[:], r1[:], r[:])
                    nc.vector.tensor_mul(d2[:], d2[:], wxa[:])
                    out8 = rpool.tile([128, S], dt.float32, tag="out8")
                    nc.vector.tensor_add(out8[:], d2[:], r[:])

                    outT = opool.tile([128, (S // 128) * 80], dt.float32, tag="outT")
                    for b in range(S // 128):
                        ft = fpool.tile([128, 80], dt.float32, tag="ft")
                        sl = slice(b * 128, (b + 1) * 128)
                        nc.tensor.matmul(ft[:], out8[:, sl], p1_t[:],
                                         start=True, stop=False)
                        nc.tensor.matmul(ft[:], wxa[:, sl], p2_t[:],
                                         start=False, stop=False)
                        nc.tensor.matmul(ft[:], wya[:, sl], p3_t[:],
                                         start=False, stop=True)
                        nc.scalar.copy(outT[:, b * 80:(b + 1) * 80], ft[:])

                    nc.sync.dma_start(out_d[ph, ch], outT[:])
    nc.compile()
    return nc


def _prep_tables(embeddings):
    # [128 maps, 16516 padded grid pts, 8 feats]
    T = np.pad(embeddings.reshape(MAPS, OFF, FEAT).astype(np.float32),
               ((0, 0), (0, 132), (0, 0)))
    v = np.arange(2 * NPAIR)
    j = v // 2
    h = v % 2
    sec = j // (NPAIR // 2)
    jj = j % (NPAIR // 2)
    colg = 2 * jj + h + sec                       # [32768]
    pp = np.arange(128)
    cm = pp // 16
    sh = (pp % 16) // 8
    ff = pp % 8
    gidx = colg[None, :] + 128 * sh[:, None]      # [128, 32768]
    tabs = []
    for k in range(NCORES):
        per_ph = []
        for ph in range(2):
            maps = np.arange(16 * k + 8 * ph, 16 * k + 8 * ph + 8)
            tab = T[maps[cm][:, None], gidx, ff[:, None]]
            per_ph.append(np.ascontiguousarray(tab, dtype=np.float32))
        tabs.append(per_ph)
    return tabs


def _prep_idx_weights(inputs):
    # inputs [B, 128, 2] -> per NC, per phase: idx [128, 64*NCH] i16,
    # wxa/wya [128, B] f32 (replicated across each 16-partition group)
    x = (inputs[..., 0].astype(np.float32) * np.float32(RES - 1)).astype(np.float32)
    y = (inputs[..., 1].astype(np.float32) * np.float32(RES - 1)).astype(np.float32)
    xi = x.astype(np.int32).astype(np.int64)          # trunc, matches reference
    yi = y.astype(np.int32).astype(np.int64)
    xf = x - xi.astype(np.float32)
    yf = y - yi.astype(np.float32)
    gg = xi * RES + yi
    par = yi & 1
    idx = ((gg - par) >> 1) + par * (NPAIR // 2)  # [B, 128] int
    out = []
    for k in range(NCORES):
        per_ph = []
        for ph in range(2):
            m0 = 16 * k + 8 * ph
            idx_m = idx[:, m0:m0 + 8]             # [B, 8]
            # wrapped: partition 16m+jj slot (ch*64 + t) <- sample 16*t+jj of chunk ch
            iw = idx_m.reshape(NCH, 64, 16, 8)    # [ch, t, jj, m]
            iw = iw.transpose(3, 2, 0, 1).reshape(8, 16, NCH * 64)
            iw = iw.reshape(128, NCH * 64, order="C")  # p = m*16+jj
            wx = np.repeat(xf[:, m0:m0 + 8].T, 16, axis=0)  # [128, B]
            wy = np.repeat(yf[:, m0:m0 + 8].T, 16, axis=0)
            per_ph.append((np.ascontiguousarray(iw.astype(np.int16)),
                           np.ascontiguousarray(wx),
                           np.ascontiguousarray(wy)))
        out.append(per_ph)
    return out


def _selectors():
    p1 = np.zeros((128, 80), np.float32)
    p2 = np.zeros((128, 80), np.float32)
    p3 = np.zeros((128, 80), np.float32)
    for p in range(128):
        m, q = p // 16, p % 16
        if q < 8:
            p1[p, m * 10 + q] = 1.0
        if q == 0:
            p2[p, m * 10 + 8] = 1.0
            p3[p, m * 10 + 9] = 1.0
    return p1, p2, p3


def build_in_maps(inputs: np.ndarray, embeddings: np.ndarray):
    if "nc" not in _cache:
        _cache["nc"] = _build_program()
    nc = _cache["nc"]

    inputs = np.asarray(inputs, dtype=np.float32)
    embeddings = np.asarray(embeddings, dtype=np.float32)
    tabs = _prep_tables(embeddings)
    iw = _prep_idx_weights(inputs)
    p1, p2, p3 = _selectors()

    in_maps = []
    for k in range(NCORES):
        m = {"p1": p1, "p2": p2, "p3": p3}
        for ph in range(2):
            ix, wx, wy = iw[k][ph]
            m[f"emb{ph}"] = tabs[k][ph]
            m[f"idx{ph}"] = ix
            m[f"wxa{ph}"] = wx
            m[f"wya{ph}"] = wy
        in_maps.append(m)
    return nc, in_maps


def kernel(inputs: np.ndarray, embeddings: np.ndarray) -> np.ndarray:
    from concourse.bass_utils import run_bass_kernel_spmd

    nc, in_maps = build_in_maps(inputs, embeddings)
    res = run_bass_kernel_spmd(nc, in_maps, core_ids=list(range(NCORES)))
    out = np.empty((B, MAPS, FEAT + 2), np.float32)
    for k in range(NCORES):
        st = res.results[k]["out"].reshape(2, NCH, 128, S // 128, PH_M, FEAT + 2)
        # out[ch*S + b*128 + p, 16k + ph*8 + m, f] = st[ph, ch, p, b, m, f]
        o = st.transpose(1, 3, 2, 0, 4, 5).reshape(B, MP_NC, FEAT + 2)
        out[:, 16 * k:16 * k + 16, :] = o
    return out

